# revision 11
# baseline (speedup 1.0000x reference)
"""Gated attention layer on 8 Trainium2 NeuronCores (Bass/Tile).

Reference (per batch b):
    temp  = einsum('qd,cd->qc', query, context)         # [512, 2048]
    alpha = softmax(temp, axis=q)                       # over the 512 axis
    awq   = einsum('qd,qc->cd', query, alpha)           # [2048, 768]
    out   = context * awq

Sharding: data-parallel over batch (B=8 -> one batch per core).

Per-core kernel strategy:
  - Load query natural [512,768] (+ a ones column -> [512,769]) and context
    natural [2048,768] into SBUF.
  - PE-transpose both to get qT [768,512] and cT [768,2048] for matmul 1
    (contraction over d must run along partitions).
  - matmul1 per c-chunk (512 cols): temp[q,c] in PSUM, q on partitions.
  - Softmax over q (the partition axis) per chunk: a single scalar shift
    per 512-column chunk (max over the whole chunk, computed via free-axis
    reduce_max + gpsimd partition_all_reduce) keeps exp() in range; the
    shift cancels exactly in the normalization, so the result matches the
    per-column-max softmax to fp32 rounding.
  - exp() goes PSUM->SBUF on the scalar engine with bias = -chunkmax.
  - matmul2 uses e[q,c] slices as stationary and query-natural as moving;
    the appended ones column makes output col 768 the softmax denominator
    per c row, i.e. already laid out per-partition for cheap scaling.
  - Epilogue fuses (awq_raw * 1/denom) * context in one DVE op.

Matmul dtype mode: "f32" (exact, 4 cyc/row) or "f32r" (1 cyc/row at N>=256).
"""

import os
import sys

import numpy as np

for _p in ("/opt/trn_rl_repo", "/root/.axon_site/_ro/trn_rl_repo"):
    if os.path.isdir(_p) and _p not in sys.path:
        sys.path.append(_p)

import concourse.bass as bass
import concourse.tile as tile
from concourse import bacc, bass_isa, masks, mybir
from concourse.bass_utils import run_bass_kernel_spmd

# ----------------------------------------------------------------------------
# Problem constants (hardcoded per spec: B=8, Lq=512, Lc=2048, D=768, fp32)
B = 8
LQ = 512
LC = 2048
D = 768
P = 128
NQT = LQ // P          # 4 query row-tiles
NCT = LC // P          # 16 context row-tiles
NDT = D // P           # 6 d tiles
CHUNK = 512            # c columns per softmax chunk
NCH = LC // CHUNK      # 4 chunks
CT_PER_CH = CHUNK // P  # 4 c-tiles per chunk

MM_MODE = os.environ.get("BASS_GATED_MM_MODE", "f32r")

F32 = mybir.dt.float32
F32R = mybir.dt.float32r
# Matmul operand tiles carry this dtype; producers (DVE copy / ACT exp)
# round into it, which is what the walrus BIR verifier requires for fp32r.
MM_DT = F32R if MM_MODE == "f32r" else F32


def build_program():
    nc = bacc.Bacc(trn_type="TRN2", target_bir_lowering=False, debug=False)

    ctx_d = nc.dram_tensor("context_emb", [LC, D], F32, kind="ExternalInput").ap()
    q_d = nc.dram_tensor("query_emb", [LQ, D], F32, kind="ExternalInput").ap()
    out_d = nc.dram_tensor("out", [LC, D], F32, kind="ExternalOutput").ap()

    ctx_t = ctx_d.rearrange("(ct p) d -> ct p d", p=P)
    q_t = q_d.rearrange("(qt p) d -> qt p d", p=P)
    out_t = out_d.rearrange("(ct p) d -> ct p d", p=P)

    with tile.TileContext(nc) as tc:
        with (
            tc.tile_pool(name="const", bufs=1) as pool_const,
            tc.tile_pool(name="qn", bufs=1) as pool_qn,
            tc.tile_pool(name="qT", bufs=1) as pool_qT,
            tc.tile_pool(name="cn", bufs=1) as pool_cn,
            tc.tile_pool(name="cT", bufs=1) as pool_cT,
            tc.tile_pool(name="e", bufs=1) as pool_e,
            tc.tile_pool(name="stats", bufs=2) as pool_stats,
            tc.tile_pool(name="osb", bufs=4) as pool_out,
            tc.tile_pool(name="ppmm1", bufs=4, space="PSUM") as pp_mm1,
            tc.tile_pool(name="pptr", bufs=2, space="PSUM") as pp_tr,
            tc.tile_pool(name="ppmm2", bufs=1, space="PSUM") as pp_mm2,
        ):
            ident = pool_const.tile([P, P], F32, tag="ident")
            masks.make_identity(nc, ident[:])
            c_off = pool_const.tile([P, 1], F32, tag="c_off")
            nc.gpsimd.memset(c_off[:], 60.0)

            # --- persistent SBUF tiles -----------------------------------
            qn = [pool_qn.tile([P, D + 2], F32, tag=f"q{qt}", name=f"qn{qt}")
                  for qt in range(NQT)]
            # rounded copy of qn for matmul2's moving operand
            qr = [pool_qn.tile([P, D + 2], MM_DT, tag=f"qr{qt}", name=f"qr{qt}")
                  for qt in range(NQT)]
            qT = [pool_qT.tile([P, LQ], MM_DT, tag=f"d{dt}", name=f"qT{dt}")
                  for dt in range(NDT)]
            cn = [pool_cn.tile([P, D], F32, tag=f"c{ct}", name=f"cn{ct}")
                  for ct in range(NCT)]
            # cT[dt][j]: [128(d), 512(c)] for chunk j
            cT = [[pool_cT.tile([P, CHUNK], MM_DT, tag=f"t{dt}_{j}",
                                name=f"cT{dt}_{j}")
                   for j in range(NCH)] for dt in range(NDT)]
            # e[qt][j]: exp'd logits [128(q), 512(c)]
            e = [[pool_e.tile([P, CHUNK], MM_DT, tag=f"e{qt}_{j}",
                              name=f"e{qt}_{j}")
                  for j in range(NCH)] for qt in range(NQT)]

            # --- query load + ones column + qT transposes ----------------
            for qt in range(NQT):
                nc.sync.dma_start(qn[qt][:, 0:D], q_t[qt])
                nc.gpsimd.memset(qn[qt][:, D:D + 2], 1.0)
                nc.vector.tensor_copy(qr[qt][:], qn[qt][:])
            for qt in range(NQT):
                for dt in range(NDT):
                    pt = pp_tr.tile([P, P], F32, tag="tr", name="ptq")
                    nc.tensor.transpose(pt[:], qn[qt][:, dt * P:(dt + 1) * P],
                                        ident[:])
                    nc.vector.tensor_copy(qT[dt][:, qt * P:(qt + 1) * P], pt[:])

            def load_and_transpose_chunk(j):
                for k in range(CT_PER_CH):
                    ct = j * CT_PER_CH + k
                    nc.sync.dma_start(cn[ct][:], ctx_t[ct])
                for k in range(CT_PER_CH):
                    ct = j * CT_PER_CH + k
                    for dt in range(NDT):
                        pt = pp_tr.tile([P, P], F32, tag="tr", name="ptc")
                        nc.tensor.transpose(
                            pt[:], cn[ct][:, dt * P:(dt + 1) * P], ident[:])
                        nc.vector.tensor_copy(
                            cT[dt][j][:, k * P:(k + 1) * P], pt[:])

            load_and_transpose_chunk(0)

            for j in range(NCH):
                # --- matmul 1: temp[q, c-chunk] in PSUM ------------------
                pieces = []
                stat = pool_stats.tile([P, NQT], F32, tag="stat", name="stat")
                for qt in range(NQT):
                    pp = pp_mm1.tile([P, CHUNK], F32, tag="mm1", name="tempp")
                    for dt in range(NDT):
                        nc.tensor.matmul(
                            pp[:],
                            qT[dt][:, qt * P:(qt + 1) * P],
                            cT[dt][j][:],
                            start=(dt == 0),
                            stop=(dt == NDT - 1),
                        )
                    nc.vector.reduce_max(stat[:, qt:qt + 1], pp[:],
                                         axis=mybir.AxisListType.X)
                    pieces.append(pp)

                # --- chunk max -> -max broadcast to all partitions -------
                m1 = pool_stats.tile([P, 1], F32, tag="m1", name="m1")
                nc.vector.reduce_max(m1[:], stat[:], axis=mybir.AxisListType.X)
                mall = pool_stats.tile([P, 1], F32, tag="mall", name="mall")
                nc.gpsimd.partition_all_reduce(
                    mall[:], m1[:], channels=P, reduce_op=bass_isa.ReduceOp.max)
                # negm = OFFSET - chunkmax.  The +OFFSET keeps weak columns'
                # exp() values well inside normal fp32 range (their
                # numerator AND denominator scale by e^OFFSET, which cancels
                # in the normalization). Without it, columns whose max sits
                # ~40 below the chunk max get flushed to zero -> 0/0 NaN.
                negm = pool_stats.tile([P, 1], F32, tag="negm", name="negm")
                nc.vector.tensor_sub(negm[:], c_off[:], mall[:])

                # --- exp: PSUM -> SBUF on ACT ----------------------------
                for qt in range(NQT):
                    nc.scalar.activation(
                        e[qt][j][:], pieces[qt][:],
                        mybir.ActivationFunctionType.Exp,
                        bias=negm[:], scale=1.0)

                # prefetch + transpose next chunk's context while exp runs
                if j + 1 < NCH:
                    load_and_transpose_chunk(j + 1)

                # --- matmul 2 + epilogue per c-tile ----------------------
                for k in range(CT_PER_CH):
                    ct = j * CT_PER_CH + k
                    po = pp_mm2.tile([P, D + 2], F32, tag="mm2", name="awqp")
                    for (lo, w) in ((0, CHUNK), (CHUNK, D + 2 - CHUNK)):
                        for qt in range(NQT):
                            nc.tensor.matmul(
                                po[:, lo:lo + w],
                                e[qt][j][:, k * P:(k + 1) * P],
                                qr[qt][:, lo:lo + w],
                                start=(qt == 0),
                                stop=(qt == NQT - 1),
                            )
                    rden = pool_stats.tile([P, 1], F32, tag="rden", name="rden")
                    nc.vector.reciprocal(rden[:], po[:, D:D + 1])
                    osb = pool_out.tile([P, D], F32, tag="osb", name="osb")
                    nc.vector.scalar_tensor_tensor(
                        osb[:], po[:, 0:D], rden[:], cn[ct][:],
                        op0=mybir.AluOpType.mult, op1=mybir.AluOpType.mult)
                    nc.sync.dma_start(out_t[ct], osb[:])

    nc.compile()
    return nc


_PROG = None


def _get_prog():
    global _PROG
    if _PROG is None:
        _PROG = build_program()
    return _PROG


def kernel(context_emb, query_emb, **_ignored):
    context_emb = np.ascontiguousarray(np.asarray(context_emb, dtype=np.float32))
    query_emb = np.ascontiguousarray(np.asarray(query_emb, dtype=np.float32))
    assert context_emb.shape == (B, LC, D), context_emb.shape
    assert query_emb.shape == (B, LQ, D), query_emb.shape

    nc = _get_prog()
    in_maps = [
        {"context_emb": context_emb[b], "query_emb": query_emb[b]}
        for b in range(B)
    ]
    res = run_bass_kernel_spmd(nc, in_maps, core_ids=list(range(B)))
    return np.stack([res.results[b]["out"] for b in range(B)], axis=0)


# revision 14
# speedup vs baseline: 1.0226x; 1.0226x over previous
"""Gated attention layer on 8 Trainium2 NeuronCores (Bass/Tile).

Reference (per batch b):
    temp  = einsum('qd,cd->qc', query, context)         # [512, 2048]
    alpha = softmax(temp, axis=q)                       # over the 512 axis
    awq   = einsum('qd,qc->cd', query, alpha)           # [2048, 768]
    out   = context * awq

Sharding: data-parallel over batch (B=8 -> one batch per core).

Per-core kernel strategy:
  - Load query natural [512,768] (+ a ones column -> [512,769]) and context
    natural [2048,768] into SBUF.
  - PE-transpose both to get qT [768,512] and cT [768,2048] for matmul 1
    (contraction over d must run along partitions).
  - matmul1 per c-chunk (512 cols): temp[q,c] in PSUM, q on partitions.
  - Softmax over q (the partition axis) per chunk: a single scalar shift
    per 512-column chunk (max over the whole chunk, computed via free-axis
    reduce_max + gpsimd partition_all_reduce) keeps exp() in range; the
    shift cancels exactly in the normalization, so the result matches the
    per-column-max softmax to fp32 rounding.
  - exp() goes PSUM->SBUF on the scalar engine with bias = -chunkmax.
  - matmul2 uses e[q,c] slices as stationary and query-natural as moving;
    the appended ones column makes output col 768 the softmax denominator
    per c row, i.e. already laid out per-partition for cheap scaling.
  - Epilogue fuses (awq_raw * 1/denom) * context in one DVE op.

Matmul dtype mode: "f32" (exact, 4 cyc/row) or "f32r" (1 cyc/row at N>=256).
"""

import os
import sys

import numpy as np

for _p in ("/opt/trn_rl_repo", "/root/.axon_site/_ro/trn_rl_repo"):
    if os.path.isdir(_p) and _p not in sys.path:
        sys.path.append(_p)

import concourse.bass as bass
import concourse.tile as tile
from concourse import bacc, bass_isa, masks, mybir
from concourse.bass_utils import run_bass_kernel_spmd

# ----------------------------------------------------------------------------
# Problem constants (hardcoded per spec: B=8, Lq=512, Lc=2048, D=768, fp32)
B = 8
LQ = 512
LC = 2048
D = 768
P = 128
NQT = LQ // P          # 4 query row-tiles
NCT = LC // P          # 16 context row-tiles
NDT = D // P           # 6 d tiles
CHUNK = 512            # c columns per softmax chunk
NCH = LC // CHUNK      # 4 chunks
CT_PER_CH = CHUNK // P  # 4 c-tiles per chunk

MM_MODE = os.environ.get("BASS_GATED_MM_MODE", "f32r")

F32 = mybir.dt.float32
F32R = mybir.dt.float32r
# Matmul operand tiles carry this dtype; producers (DVE copy / ACT exp)
# round into it, which is what the walrus BIR verifier requires for fp32r.
MM_DT = F32R if MM_MODE == "f32r" else F32
BF16 = mybir.dt.bfloat16
# matmul2 operand dtype: alpha is post-softmax, bf16 rounding there only
# costs ~0.4% relative on the output (vs 50%+ if used for logits).
MM2_DT = BF16 if os.environ.get("BASS_GATED_MM2", "bf16") == "bf16" else MM_DT


def build_program():
    nc = bacc.Bacc(trn_type="TRN2", target_bir_lowering=False, debug=False)

    ctx_d = nc.dram_tensor("context_emb", [LC, D], F32, kind="ExternalInput").ap()
    q_d = nc.dram_tensor("query_emb", [LQ, D], F32, kind="ExternalInput").ap()
    out_d = nc.dram_tensor("out", [LC, D], F32, kind="ExternalOutput").ap()

    ctx_t = ctx_d.rearrange("(ct p) d -> ct p d", p=P)
    q_t = q_d.rearrange("(qt p) d -> qt p d", p=P)
    out_t = out_d.rearrange("(ct p) d -> ct p d", p=P)

    with tile.TileContext(nc) as tc:
        with (
            tc.tile_pool(name="const", bufs=1) as pool_const,
            tc.tile_pool(name="qn", bufs=1) as pool_qn,
            tc.tile_pool(name="qT", bufs=1) as pool_qT,
            tc.tile_pool(name="cn", bufs=1) as pool_cn,
            tc.tile_pool(name="cT", bufs=1) as pool_cT,
            tc.tile_pool(name="e", bufs=1) as pool_e,
            tc.tile_pool(name="stats", bufs=2) as pool_stats,
            tc.tile_pool(name="osb", bufs=4) as pool_out,
            tc.tile_pool(name="ppmm1", bufs=4, space="PSUM") as pp_mm1,
            tc.tile_pool(name="pptr", bufs=2, space="PSUM") as pp_tr,
            tc.tile_pool(name="ppmm2", bufs=1, space="PSUM") as pp_mm2,
        ):
            ident = pool_const.tile([P, P], F32, tag="ident")
            masks.make_identity(nc, ident[:])
            c_off = pool_const.tile([P, 1], F32, tag="c_off")
            nc.gpsimd.memset(c_off[:], 60.0)

            # --- persistent SBUF tiles -----------------------------------
            qn = [pool_qn.tile([P, D + 2], F32, tag=f"q{qt}", name=f"qn{qt}")
                  for qt in range(NQT)]
            # rounded copy of qn for matmul2's moving operand
            qr = [pool_qn.tile([P, D + 2], MM2_DT, tag=f"qr{qt}", name=f"qr{qt}")
                  for qt in range(NQT)]
            qT = [pool_qT.tile([P, LQ], MM_DT, tag=f"d{dt}", name=f"qT{dt}")
                  for dt in range(NDT)]
            cn = [pool_cn.tile([P, D], F32, tag=f"c{ct}", name=f"cn{ct}")
                  for ct in range(NCT)]
            # cT[dt][j]: [128(d), 512(c)] for chunk j
            cT = [[pool_cT.tile([P, CHUNK], MM_DT, tag=f"t{dt}_{j}",
                                name=f"cT{dt}_{j}")
                   for j in range(NCH)] for dt in range(NDT)]
            # e[qt][j]: exp'd logits [128(q), 512(c)]
            e = [[pool_e.tile([P, CHUNK], MM2_DT, tag=f"e{qt}_{j}",
                              name=f"e{qt}_{j}")
                  for j in range(NCH)] for qt in range(NQT)]

            # --- query load + ones column + qT transposes ----------------
            for qt in range(NQT):
                nc.sync.dma_start(qn[qt][:, 0:D], q_t[qt])
                nc.gpsimd.memset(qn[qt][:, D:D + 2], 1.0)
                nc.vector.tensor_copy(qr[qt][:], qn[qt][:])
            for qt in range(NQT):
                for dt in range(NDT):
                    pt = pp_tr.tile([P, P], F32, tag="tr", name="ptq")
                    nc.tensor.transpose(pt[:], qn[qt][:, dt * P:(dt + 1) * P],
                                        ident[:])
                    nc.vector.tensor_copy(qT[dt][:, qt * P:(qt + 1) * P], pt[:])

            def load_and_transpose_chunk(j):
                for k in range(CT_PER_CH):
                    ct = j * CT_PER_CH + k
                    nc.sync.dma_start(cn[ct][:], ctx_t[ct])
                for k in range(CT_PER_CH):
                    ct = j * CT_PER_CH + k
                    for dt in range(NDT):
                        pt = pp_tr.tile([P, P], F32, tag="tr", name="ptc")
                        nc.tensor.transpose(
                            pt[:], cn[ct][:, dt * P:(dt + 1) * P], ident[:])
                        nc.vector.tensor_copy(
                            cT[dt][j][:, k * P:(k + 1) * P], pt[:])

            load_and_transpose_chunk(0)

            for j in range(NCH):
                # --- matmul 1: temp[q, c-chunk] in PSUM ------------------
                pieces = []
                stat = pool_stats.tile([P, NQT], F32, tag="stat", name="stat")
                for qt in range(NQT):
                    pp = pp_mm1.tile([P, CHUNK], F32, tag="mm1", name="tempp")
                    for dt in range(NDT):
                        nc.tensor.matmul(
                            pp[:],
                            qT[dt][:, qt * P:(qt + 1) * P],
                            cT[dt][j][:],
                            start=(dt == 0),
                            stop=(dt == NDT - 1),
                        )
                    nc.vector.reduce_max(stat[:, qt:qt + 1], pp[:],
                                         axis=mybir.AxisListType.X)
                    pieces.append(pp)

                # --- chunk max -> -max broadcast to all partitions -------
                m1 = pool_stats.tile([P, 1], F32, tag="m1", name="m1")
                nc.vector.reduce_max(m1[:], stat[:], axis=mybir.AxisListType.X)
                mall = pool_stats.tile([P, 1], F32, tag="mall", name="mall")
                nc.gpsimd.partition_all_reduce(
                    mall[:], m1[:], channels=P, reduce_op=bass_isa.ReduceOp.max)
                # negm = OFFSET - chunkmax.  The +OFFSET keeps weak columns'
                # exp() values well inside normal fp32 range (their
                # numerator AND denominator scale by e^OFFSET, which cancels
                # in the normalization). Without it, columns whose max sits
                # ~40 below the chunk max get flushed to zero -> 0/0 NaN.
                negm = pool_stats.tile([P, 1], F32, tag="negm", name="negm")
                nc.vector.tensor_sub(negm[:], c_off[:], mall[:])

                # --- exp: PSUM -> SBUF on ACT ----------------------------
                for qt in range(NQT):
                    nc.scalar.activation(
                        e[qt][j][:], pieces[qt][:],
                        mybir.ActivationFunctionType.Exp,
                        bias=negm[:], scale=1.0)

                # prefetch + transpose next chunk's context while exp runs
                if j + 1 < NCH:
                    load_and_transpose_chunk(j + 1)

                # --- matmul 2 + epilogue per c-tile ----------------------
                for k in range(CT_PER_CH):
                    ct = j * CT_PER_CH + k
                    po = pp_mm2.tile([P, D + 2], F32, tag="mm2", name="awqp")
                    for (lo, w) in ((0, CHUNK), (CHUNK, D + 2 - CHUNK)):
                        for qt in range(NQT):
                            nc.tensor.matmul(
                                po[:, lo:lo + w],
                                e[qt][j][:, k * P:(k + 1) * P],
                                qr[qt][:, lo:lo + w],
                                start=(qt == 0),
                                stop=(qt == NQT - 1),
                            )
                    rden = pool_stats.tile([P, 1], F32, tag="rden", name="rden")
                    nc.vector.reciprocal(rden[:], po[:, D:D + 1])
                    osb = pool_out.tile([P, D], F32, tag="osb", name="osb")
                    nc.vector.scalar_tensor_tensor(
                        osb[:], po[:, 0:D], rden[:], cn[ct][:],
                        op0=mybir.AluOpType.mult, op1=mybir.AluOpType.mult)
                    nc.sync.dma_start(out_t[ct], osb[:])

    nc.compile()
    return nc


_PROG = None


def _get_prog():
    global _PROG
    if _PROG is None:
        _PROG = build_program()
    return _PROG


def kernel(context_emb, query_emb, **_ignored):
    context_emb = np.ascontiguousarray(np.asarray(context_emb, dtype=np.float32))
    query_emb = np.ascontiguousarray(np.asarray(query_emb, dtype=np.float32))
    assert context_emb.shape == (B, LC, D), context_emb.shape
    assert query_emb.shape == (B, LQ, D), query_emb.shape

    nc = _get_prog()
    in_maps = [
        {"context_emb": context_emb[b], "query_emb": query_emb[b]}
        for b in range(B)
    ]
    res = run_bass_kernel_spmd(nc, in_maps, core_ids=list(range(B)))
    return np.stack([res.results[b]["out"] for b in range(B)], axis=0)
